# revision 1
# baseline (speedup 1.0000x reference)
"""GQA attention kernel for 8 Trainium2 NeuronCores.

Problem: B=2, N=2048, D=2048, H=32 heads, G=8 KV groups, head_dim=64, RoPE,
causal mask, fused QKV/output projections.

Sharding: one (batch, group-pair) unit per core — core c handles batch c//4
and KV groups {2*(c%4), 2*(c%4)+1} (8 query heads). Each core computes a
partial output projection (its heads' rows of Wo); the host sums the 4
partials per batch.

Per-core pipeline (all matmuls in bf16, fp32 accumulate):
  x --cast DMA--> xbf (DRAM, bf16) --xbar-transpose DMA--> xT [din, tok]
  QKV projections (lhsT = xT blocks), RoPE in natural layout on DVE,
  PE-transpose q/k to q^T/k^T [d, tok], then per head, key-block-major:
    scores^T[m] = k^T_m.T @ q^T  (PSUM) -> exp on ACT -> attn^T (bf16 SBUF)
    causal: skip key blocks above the diagonal; triangular mask on diag block
    ctx^T += [v_m | 1].T @ attn^T_m  -> row 64 = softmax denominators
  normalize ctx^T by broadcast reciprocal denominators, out = ctx^T.T @ Wo.
"""

import numpy as np

import concourse.bass as bass
import concourse.bacc as bacc
import concourse.mybir as mybir
import concourse.tile as tile
from concourse.bass_utils import run_bass_kernel_spmd
from concourse.masks import make_identity, make_upper_triangular

F32 = mybir.dt.float32
BF16 = mybir.dt.bfloat16

N = 2048          # sequence length
D = 2048          # model dim
HD = 64           # head dim
QF = 512          # q features per core (8 heads)
KF = 128          # k/v features per core (2 groups)
NT = N // 128     # token blocks
KC = D // 128     # contraction chunks
SCALE = 1.0 / 8.0  # 1/sqrt(HD)


def _build_program():
    nc = bacc.Bacc("TRN2", debug=False, target_bir_lowering=False)

    x_d = nc.dram_tensor("x", [N, D], F32, kind="ExternalInput")
    cos_d = nc.dram_tensor("cos", [N, HD], F32, kind="ExternalInput")
    sin_d = nc.dram_tensor("sin", [N, HD], F32, kind="ExternalInput")
    wq_d = nc.dram_tensor("wq", [D, QF], F32, kind="ExternalInput")
    wk_d = nc.dram_tensor("wk", [D, KF], F32, kind="ExternalInput")
    wv_d = nc.dram_tensor("wv", [D, KF], F32, kind="ExternalInput")
    wo_d = nc.dram_tensor("wo", [QF, D], F32, kind="ExternalInput")
    out_d = nc.dram_tensor("out", [N, D], F32, kind="ExternalOutput")

    with tile.TileContext(nc) as tc:
        with tc.tile_pool(name="persist", bufs=1) as pp:
            # persistent SBUF: q^T/k^T, [v|1], ctx^T, wo, constants
            qT = [pp.tile([128, N], BF16, name=f"qT{t}") for t in range(4)]
            kT = pp.tile([128, N], BF16, name="kT")
            vo = [pp.tile([128, NT, HD + 1], BF16, name=f"vo{g}") for g in range(2)]
            ctxT = [pp.tile([128, N], BF16, name=f"ctxT{k}") for k in range(4)]
            wo_sb = pp.tile([128, 4, N], BF16, name="wo_sb")
            ident = pp.tile([128, 128], BF16, name="ident")
            maskt = pp.tile([128, 128], BF16, name="maskt")

            make_identity(nc, ident)
            make_upper_triangular(nc, maskt, val=1.0, diag=True)
            for g in range(2):
                nc.vector.memset(vo[g][:, :, HD:HD + 1], 1.0)

            nc.gpsimd.dma_start(
                wo_sb[:], wo_d[:].rearrange("(ko ki) n -> ki ko n", ki=128))

            # ---------------- phase A: x^T + projections + rope ----------
            with tc.tile_pool(name="phaseA", bufs=1) as pa, \
                 tc.tile_pool(name="dram", bufs=1, space="DRAM") as dp, \
                 tc.tile_pool(name="ps_q", bufs=2, space="PSUM") as ps_q, \
                 tc.tile_pool(name="ps_kv", bufs=2, space="PSUM") as ps_kv, \
                 tc.tile_pool(name="ps_tr", bufs=2, space="PSUM") as ps_tr, \
                 tc.tile_pool(name="ropetmp", bufs=6) as rtp:

                xbf = dp.tile([N, D], BF16, name="xbf")
                xT = [pa.tile([128, N], BF16, name=f"xT{kc}")
                      for kc in range(KC)]
                wq_sb = pa.tile([128, KC, QF], BF16, name="wq_sb")
                wkv_sb = pa.tile([128, KC, 2 * KF], BF16, name="wkv_sb")
                cos_sb = pa.tile([128, NT, HD], F32, name="cos_sb")
                sin_sb = pa.tile([128, NT, HD], F32, name="sin_sb")
                q_rope = pa.tile([128, NT, QF], BF16, name="q_rope")
                k_rope = pa.tile([128, NT, KF], BF16, name="k_rope")

                nc.gpsimd.dma_start(
                    wq_sb[:], wq_d[:].rearrange("(ko ki) n -> ki ko n", ki=128))
                nc.gpsimd.dma_start(
                    wkv_sb[:, :, 0:KF],
                    wk_d[:].rearrange("(ko ki) n -> ki ko n", ki=128))
                nc.gpsimd.dma_start(
                    wkv_sb[:, :, KF:2 * KF],
                    wv_d[:].rearrange("(ko ki) n -> ki ko n", ki=128))
                nc.sync.dma_start(
                    cos_sb[:], cos_d[:].rearrange("(t p) d -> p t d", p=128))
                nc.sync.dma_start(
                    sin_sb[:], sin_d[:].rearrange("(t p) d -> p t d", p=128))

                # cast x to bf16 in DRAM (one DMA: the xbar-transpose
                # instruction has very few sync-wait slots, so it can only
                # depend on a single producer), then xbar-transpose each
                # [2048, 128] column block into xT
                nc.gpsimd.dma_start(xbf[:], x_d[:])
                for kc in range(KC):
                    nc.sync.dma_start_transpose(
                        xT[kc][:], xbf[:, kc * 128:(kc + 1) * 128])

                def rope(ps, cos_b, sin_b, out_v, ab_shape):
                    """ps 4D view [128, *ab, 2, 32]; cos_b/sin_b broadcast
                    [128, *ab, 32]; out_v same 4D view layout as ps."""
                    q1 = ps[..., 0, :]
                    q2 = ps[..., 1, :]
                    c1 = cos_b[0]
                    c2 = cos_b[1]
                    s1 = sin_b[0]
                    s2 = sin_b[1]
                    ta = rtp.tile([128] + ab_shape + [32], F32, name="rt", tag="rt")
                    tb = rtp.tile([128] + ab_shape + [32], F32, name="rt", tag="rt")
                    nc.vector.tensor_mul(ta[:], q1, c1)
                    nc.vector.tensor_mul(tb[:], q2, s1)
                    nc.vector.tensor_sub(out_v[..., 0, :], ta[:], tb[:])
                    tc_ = rtp.tile([128] + ab_shape + [32], F32, name="rt", tag="rt")
                    td = rtp.tile([128] + ab_shape + [32], F32, name="rt", tag="rt")
                    nc.vector.tensor_mul(tc_[:], q2, c2)
                    nc.vector.tensor_mul(td[:], q1, s2)
                    nc.vector.tensor_add(out_v[..., 1, :], tc_[:], td[:])

                for tb_i in range(NT):
                    psq = ps_q.tile([128, QF], F32, name="psq", tag="psq")
                    pskv = ps_kv.tile([128, 2 * KF], F32, name="pskv", tag="pskv")
                    for kc in range(KC):
                        lhsT = xT[kc][:, tb_i * 128:(tb_i + 1) * 128]
                        st = kc == 0
                        sp = kc == KC - 1
                        nc.tensor.matmul(psq[:], lhsT, wq_sb[:, kc, :],
                                         start=st, stop=sp)
                        nc.tensor.matmul(pskv[:], lhsT, wkv_sb[:, kc, :],
                                         start=st, stop=sp)

                    # --- RoPE Q: psq cols = a*256 + b*64 + h*32 + j
                    #     out cols = b*128 + a*64 + h*32 + j (head pairs
                    #     (t, t+4) adjacent for the transpose step)
                    psq_v = psq[:].rearrange("p (a b h j) -> p a b h j",
                                             a=2, b=4, h=2)
                    out_v = q_rope[:, tb_i, :].rearrange(
                        "p (b a h j) -> p a b h j", b=4, a=2, h=2)
                    cs = cos_sb[:, tb_i, :]
                    sn = sin_sb[:, tb_i, :]

                    def bcq(apv):
                        return apv.unsqueeze(1).unsqueeze(1).broadcast_to(
                            (128, 2, 4, 32))

                    rope(psq_v,
                         (bcq(cs[:, 0:32]), bcq(cs[:, 32:64])),
                         (bcq(sn[:, 0:32]), bcq(sn[:, 32:64])),
                         out_v, [2, 4])

                    # --- RoPE K: cols = g*64 + h*32 + j (no interleave)
                    psk_v = pskv[:, 0:KF].rearrange("p (g h j) -> p g h j",
                                                    g=2, h=2)
                    outk_v = k_rope[:, tb_i, :].rearrange(
                        "p (g h j) -> p g h j", g=2, h=2)

                    def bck(apv):
                        return apv.unsqueeze(1).broadcast_to((128, 2, 32))

                    rope(psk_v,
                         (bck(cs[:, 0:32]), bck(cs[:, 32:64])),
                         (bck(sn[:, 0:32]), bck(sn[:, 32:64])),
                         outk_v, [2])

                    # --- V -> bf16 SBUF with ones column
                    for g in range(2):
                        nc.scalar.copy(vo[g][:, tb_i, 0:HD],
                                       pskv[:, KF + g * 64:KF + (g + 1) * 64])

                    # --- PE transposes: q_rope/k_rope -> qT/kT
                    for t in range(4):
                        ptr = ps_tr.tile([128, 128], BF16, name="ptr", tag="ptr")
                        nc.tensor.transpose(
                            ptr[:], q_rope[:, tb_i, t * 128:(t + 1) * 128],
                            ident[:])
                        nc.vector.tensor_copy(
                            qT[t][:, tb_i * 128:(tb_i + 1) * 128], ptr[:])
                    ptrk = ps_tr.tile([128, 128], BF16, name="ptr", tag="ptr")
                    nc.tensor.transpose(ptrk[:], k_rope[:, tb_i, :], ident[:])
                    nc.scalar.copy(kT[:, tb_i * 128:(tb_i + 1) * 128], ptrk[:])

            # ---------------- phase B: attention ------------------------
            with tc.tile_pool(name="ps_sc", bufs=2, space="PSUM") as ps_sc, \
                 tc.tile_pool(name="ps_cx", bufs=1, space="PSUM") as ps_cx, \
                 tc.tile_pool(name="attnp", bufs=3) as ap_, \
                 tc.tile_pool(name="dramn", bufs=2, space="DRAM") as dnp, \
                 tc.tile_pool(name="normp", bufs=1) as np_:

                for l in range(8):          # local head
                    a, b = l // 4, l % 4
                    r0 = 64 * a
                    psx = ps_cx.tile([HD + 1, N], F32, name="psx", tag="psx")
                    for m in range(NT):
                        start_col = m * 128
                        lhs_k = kT[r0:r0 + 64, start_col:start_col + 128]
                        # spans of <=1024 columns, aligned to the 1024 grid
                        c = start_col
                        while c < N:
                            span_end = min(N, (c // 1024 + 1) * 1024)
                            w = span_end - c
                            psc = ps_sc.tile([128, 1024], F32, name="psc",
                                             tag="psc")
                            off = 0
                            while off < w:
                                nw = min(512, w - off)
                                nc.tensor.matmul(
                                    psc[:, off:off + nw], lhs_k,
                                    qT[b][r0:r0 + 64, c + off:c + off + nw],
                                    start=True, stop=True)
                                off += nw
                            at = ap_.tile([128, 1024], BF16, name="at",
                                          tag="at")
                            nc.scalar.activation(
                                at[:, :w], psc[:, :w],
                                mybir.ActivationFunctionType.Exp, scale=SCALE)
                            if c == start_col:
                                nc.vector.tensor_mul(
                                    at[:, 0:128], at[:, 0:128], maskt[:])
                            # ctx^T accumulation, chunks aligned to psx banks
                            off = 0
                            while off < w:
                                gc0 = c + off
                                nw = min(512 - gc0 % 512, w - off)
                                m_last = min(NT - 1, (gc0 + nw - 1) // 128)
                                nc.tensor.matmul(
                                    psx[:, gc0:gc0 + nw], vo[a][:, m, :],
                                    at[:, off:off + nw],
                                    start=(m == 0), stop=(m == m_last),
                                    skip_group_check=True)
                                off += nw
                            c = span_end

                    # normalize: recip of sums row, broadcast via DRAM
                    # roundtrip (partition-broadcast DMA needs a DRAM source)
                    rrow = np_.tile([65, N], F32, name="rrow", tag="rrow")
                    nc.vector.reciprocal(rrow[64:65, :], psx[64:65, :])
                    rd = dnp.tile([1, N], F32, name="rd", tag="rd")
                    nc.sync.dma_start(rd[:], rrow[64:65, :])
                    rb = np_.tile([64, N], F32, name="rb", tag="rb")
                    nc.sync.dma_start(rb[:], rd[:].to_broadcast((64, N)))
                    pk = l // 2
                    if l % 2 == 0:
                        nc.vector.tensor_mul(ctxT[pk][0:64, :], psx[0:64, :],
                                             rb[:])
                    else:
                        codd = np_.tile([64, N], BF16, name="codd", tag="codd")
                        nc.vector.tensor_mul(codd[:], psx[0:64, :], rb[:])
                        nc.sync.dma_start(ctxT[pk][64:128, :], codd[:])

            # ---------------- phase C: output projection ----------------
            with tc.tile_pool(name="ps_o", bufs=2, space="PSUM") as ps_o, \
                 tc.tile_pool(name="outp", bufs=2) as op_:
                for tb_i in range(NT):
                    pso = ps_o.tile([128, N], F32, name="pso", tag="pso")
                    for k4 in range(4):
                        lhsT = ctxT[k4][:, tb_i * 128:(tb_i + 1) * 128]
                        for nk in range(4):
                            nc.tensor.matmul(
                                pso[:, nk * 512:(nk + 1) * 512], lhsT,
                                wo_sb[:, k4, nk * 512:(nk + 1) * 512],
                                start=(k4 == 0), stop=(k4 == 3))
                    ost = op_.tile([128, N], F32, name="ost", tag="ost")
                    if tb_i % 2 == 0:
                        nc.scalar.copy(ost[:], pso[:])
                    else:
                        nc.vector.tensor_copy(ost[:], pso[:])
                    nc.sync.dma_start(
                        out_d[tb_i * 128:(tb_i + 1) * 128, :], ost[:])

    nc.compile()
    return nc


_NC_CACHE = {}


def _get_nc():
    if "nc" not in _NC_CACHE:
        _NC_CACHE["nc"] = _build_program()
    return _NC_CACHE["nc"]


def kernel(x, cos, sin, mask, Wq, Wk, Wv, Wo, _trace=False, _trace_kwargs=None):
    x = np.asarray(x, dtype=np.float32)
    cos = np.asarray(cos, dtype=np.float32)
    sin = np.asarray(sin, dtype=np.float32)
    Wq = np.asarray(Wq, dtype=np.float32)
    Wk = np.asarray(Wk, dtype=np.float32)
    Wv = np.asarray(Wv, dtype=np.float32)
    Wo = np.asarray(Wo, dtype=np.float32)

    nc = _get_nc()
    in_maps = []
    for c in range(8):
        bidx = c // 4
        p = c % 4
        in_maps.append({
            "x": np.ascontiguousarray(x[bidx]),
            "cos": cos,
            "sin": sin,
            "wq": np.ascontiguousarray(Wq[:, p * 512:(p + 1) * 512]),
            "wk": np.ascontiguousarray(Wk[:, p * 128:(p + 1) * 128]),
            "wv": np.ascontiguousarray(Wv[:, p * 128:(p + 1) * 128]),
            "wo": np.ascontiguousarray(Wo[p * 512:(p + 1) * 512, :]),
        })

    kwargs = {}
    if _trace:
        kwargs["trace"] = True
        kwargs.update(_trace_kwargs or {})
    res = run_bass_kernel_spmd(nc, in_maps, core_ids=list(range(8)), **kwargs)
    parts = [r["out"] for r in res.results]
    out = np.stack([
        parts[0] + parts[1] + parts[2] + parts[3],
        parts[4] + parts[5] + parts[6] + parts[7],
    ]).astype(np.float32)
    if _trace:
        kernel._last_result = res
    return out



# revision 28
# speedup vs baseline: 1.3846x; 1.3846x over previous
"""GQA attention kernel for 8 Trainium2 NeuronCores (v2).

Problem: B=2, N=2048, D=2048, H=32 heads, G=8 KV groups, head_dim=64, RoPE,
causal mask, fused QKV/output projections.

Sharding: one (batch, group-pair) unit per core - core c handles batch c//4
and KV groups {2*(c%4), 2*(c%4)+1} (8 query heads). Each core computes a
partial output projection; the host sums 4 partials per batch (and
transposes: the kernel emits out^T [dout, tok] in bf16).

Per-core pipeline (bf16 matmuls, fp32 accumulate):
  x --cast DMA--> xn stripes (SBUF bf16) --PE transpose--> xT [d, tok]
  Projections use stationary-swap: stationary = weight chunk [128d, 128f],
  moving = xT spans -> outputs land TRANSPOSED (qT/kT/vT [feat, tok]).
  RoPE: qrot = rotP @ qU on PE (rotP = signed rotate-half permutation), then
  qT = qU*cosT + qrot*sinT on DVE (cosT/sinT pre-transposed [hd, tok]).
  Attention per head, key-block-major, software-pipelined one tile deep;
  the causal mask is ADDED on PE via an extra matmul (maskLT^T @ I) into
  the scores PSUM; exp is split between ACT (true exp) and DVE (int16
  Schraudolph fast-exp written through a bf16 bitcast); ctx^T accumulates
  in column halves [65, 1024] whose ones-row yields softmax denominators;
  normalize = DVE approx-reciprocal + SBUF broadcast DMA + DVE multiply.
  Output proj: stationary = Wo chunk, moving = ctxT -> out^T, bf16 DMA.

Host-side: Wq columns / Wo rows are permuted so q-head chunk j holds
[head j | head j+4] - this aligns each head's qT partition base (0/64)
with its KV group's kT partition base, as the PE requires.
"""

import numpy as np

import concourse.bass as bass
import concourse.bacc as bacc
import concourse.mybir as mybir
import concourse.tile as tile
from concourse.bass_utils import run_bass_kernel_spmd
from concourse.masks import make_identity, make_upper_triangular

F32 = mybir.dt.float32
BF16 = mybir.dt.bfloat16
I16 = mybir.dt.int16

N = 2048          # sequence length
D = 2048          # model dim
HD = 64           # head dim
QF = 512          # q features per core (8 heads)
KF = 128          # k/v features per core (2 groups)
NT = N // 128     # token blocks
KC = D // 128     # contraction chunks
SCALE = 1.0 / 8.0  # 1/sqrt(HD)
MASKVAL = -480.0   # additive causal mask pre-scale (-60 post-scale)

# bf16 Schraudolph fast-exp: bits = round(x*SCALE*A + B) as int16 -> bf16
EXP_A = 128.0 / float(np.log(2.0))
EXP_B = 16256.0 - 7.0
# exp engine pattern per psc tile (cycled): False=ACT true exp, True=DVE fast
EXP_PATTERN = (False, False, False, True, True)


def _build_program(dbg=False):
    nc = bacc.Bacc("TRN2", debug=False, target_bir_lowering=False)

    x_d = nc.dram_tensor("x", [N, D], F32, kind="ExternalInput")
    cos_d = nc.dram_tensor("cos", [N, HD], F32, kind="ExternalInput")
    sin_d = nc.dram_tensor("sin", [N, HD], F32, kind="ExternalInput")
    wq_d = nc.dram_tensor("wq", [D, QF], F32, kind="ExternalInput")
    wk_d = nc.dram_tensor("wk", [D, KF], F32, kind="ExternalInput")
    wv_d = nc.dram_tensor("wv", [D, KF], F32, kind="ExternalInput")
    wo_d = nc.dram_tensor("wo", [QF, D], F32, kind="ExternalInput")
    out_d = nc.dram_tensor("out", [D, N], BF16, kind="ExternalOutput")
    if dbg:
        dbg_d = {nm: nc.dram_tensor(f"dbg_{nm}", [128, N], BF16,
                                    kind="ExternalOutput")
                 for nm in ("xt0", "qt0", "qt1", "kt", "cost", "sint",
                            "ctxt0", "ctxt1", "vo0", "vo1", "at00", "psx0",
                            "nds", "nds2", "nrb")}

    with tile.TileContext(nc) as tc:
        with tc.tile_pool(name="persist", bufs=1) as pp:
            qT = [pp.tile([128, N], BF16, name=f"qT{j}") for j in range(4)]
            kT = pp.tile([128, N], BF16, name="kT")
            vo = [pp.tile([128, NT, HD + 1], BF16, name=f"vo{g}") for g in range(2)]
            ctxT = [pp.tile([128, N], BF16, name=f"ctxT{j}") for j in range(4)]
            wq_sb = pp.tile([128, KC, QF], BF16, name="wq_sb")
            wkv_sb = pp.tile([128, KC, 2 * KF], BF16, name="wkv_sb")
            wo_sb = pp.tile([128, 4, N], BF16, name="wo_sb")
            cosT = pp.tile([128, N], BF16, name="cosT")
            sinT = pp.tile([128, N], BF16, name="sinT")
            ident = pp.tile([128, 128], BF16, name="ident")
            maskLT = pp.tile([128, 128], BF16, name="maskLT")
            rotP = pp.tile([128, 128], BF16, name="rotP")

            make_identity(nc, ident)
            # maskLT[r, key] = MASKVAL for key > r (strict upper)
            make_upper_triangular(nc, maskLT, val=MASKVAL, diag=False)

            # rotate-half as lhsT: rotP[k, k+32] = +1 (k%64<32),
            #                      rotP[k, k-32] = -1 (k%64>=32)
            nc.gpsimd.memset(rotP[:], 0.0)
            for b0 in (0, 64):
                nc.gpsimd.tensor_copy(
                    rotP[b0:b0 + 32, b0 + 32:b0 + 64],
                    ident[b0:b0 + 32, b0:b0 + 32])
                nc.gpsimd.tensor_scalar_mul(
                    rotP[b0 + 32:b0 + 64, b0:b0 + 32],
                    ident[b0 + 32:b0 + 64, b0 + 32:b0 + 64], -1.0)
            for g in range(2):
                nc.vector.memset(vo[g][:, :, HD:HD + 1], 1.0)

            # ---- DMA issue order: cs, wkv, then wq groups // x stripes ---
            with tc.tile_pool(name="phA_sb", bufs=1) as pa:
                cs_nat = pa.tile([128, NT, 2 * HD], F32, name="cs_nat")
                nc.sync.dma_start(
                    cs_nat[:, :, 0:HD],
                    cos_d[:].rearrange("(t p) d -> p t d", p=128))
                nc.sync.dma_start(
                    cs_nat[:, :, HD:2 * HD],
                    sin_d[:].rearrange("(t p) d -> p t d", p=128))
                nc.gpsimd.dma_start(
                    wkv_sb[:, :, 0:KF],
                    wk_d[:].rearrange("(ko ki) n -> ki ko n", ki=128))
                nc.gpsimd.dma_start(
                    wkv_sb[:, :, KF:2 * KF],
                    wv_d[:].rearrange("(ko ki) n -> ki ko n", ki=128))

                xn = []
                for kc in range(KC):
                    t = pa.tile([128, NT, 128], BF16, name="xn", tag="xn", bufs=4)
                    nc.gpsimd.dma_start(
                        t[:], x_d[:, kc * 128:(kc + 1) * 128]
                        .rearrange("(t p) d -> p t d", p=128))
                    xn.append(t)
                    if kc % 4 == 0:
                        kg = kc // 4
                        nc.gpsimd.dma_start(
                            wq_sb[:, kg * 4:(kg + 1) * 4, :],
                            wq_d[kg * 512:(kg + 1) * 512, :]
                            .rearrange("(ko ki) n -> ki ko n", ki=128))
                nc.gpsimd.dma_start(
                    wo_sb[:], wo_d[:].rearrange("(ko ki) n -> ki ko n", ki=128))

                cs_bf = pa.tile([128, NT, 2 * HD], BF16, name="cs_bf")
                nc.vector.tensor_copy(cs_bf[:], cs_nat[:])

                xT = [pa.tile([128, N], BF16, name=f"xT{kc}") for kc in range(KC)]

                # generic rope over one 512-col span
                def rope_span(dst_tile, row0_u, u, col0, s_off, rot_pool,
                              rot_tag="rot"):
                    """dst_tile cols [col0+s_off, col0+s_off+512) =
                    u[:, s_off:s_off+512]*cosT + (rotP@u)*sinT."""
                    rot = rot_pool.tile([128, 512], F32, name="rot", tag=rot_tag,
                                        bufs=2)
                    nc.tensor.matmul(rot[:], rotP[:], u[:, s_off:s_off + 512],
                                     start=True, stop=True)
                    c0 = col0 + s_off
                    t1 = pa.tile([128, 512], BF16, name="t1", tag="t1", bufs=2)
                    t2 = pa.tile([128, 512], BF16, name="t2", tag="t2", bufs=2)
                    nc.vector.tensor_mul(t1[:], u[:, s_off:s_off + 512],
                                         cosT[:, c0:c0 + 512])
                    nc.vector.tensor_mul(t2[:], rot[:], sinT[:, c0:c0 + 512])
                    nc.vector.tensor_add(dst_tile[:, c0:c0 + 512], t1[:], t2[:])

                # ---- psum scope 1: K (halves) + Q0 left (quarters) -------
                with tc.tile_pool(name="phA_ps1", bufs=1, space="PSUM") as pk:
                    kacc = [pk.tile([128, 1024], F32, name=f"kacc{h}")
                            for h in range(2)]
                    q0a = pk.tile([128, 512], F32, name="q0a")
                    q0b = pk.tile([128, 512], F32, name="q0b")

                    with tc.tile_pool(name="phA_tr", bufs=1, space="PSUM") as pt:
                        # cos/sin transposes (tiny)
                        for tb in range(NT):
                            ptc = pt.tile([128, 128], BF16, name="ptc", tag="tr",
                                          bufs=2)
                            nc.tensor.transpose(
                                ptc[0:2 * HD, :], cs_bf[:, tb, :], ident[:])
                            nc.vector.tensor_copy(
                                cosT[0:HD, tb * 128:(tb + 1) * 128],
                                ptc[0:HD, :])
                            nc.scalar.copy(
                                sinT[0:HD, tb * 128:(tb + 1) * 128],
                                ptc[HD:2 * HD, :])
                        nc.sync.dma_start(cosT[HD:128, :], cosT[0:HD, :])
                        nc.sync.dma_start(sinT[HD:128, :], sinT[0:HD, :])

                        # stripe loop: x transposes + K + Q0-left accumulation
                        for kc in range(KC):
                            for tb4 in range(4):
                                ptr = pt.tile([128, 512], BF16, name="ptr",
                                              tag="tr", bufs=2)
                                for i in range(4):
                                    tb = tb4 * 4 + i
                                    nc.tensor.transpose(
                                        ptr[:, i * 128:(i + 1) * 128],
                                        xn[kc][:, tb, :], ident[:])
                                cp = nc.vector.tensor_copy if tb4 % 2 == 0 \
                                    else nc.scalar.copy
                                cp(xT[kc][:, tb4 * 512:(tb4 + 1) * 512], ptr[:])
                            st = kc == 0
                            sp = kc == KC - 1
                            for h in range(2):
                                for s in range(2):
                                    nc.tensor.matmul(
                                        kacc[h][:, s * 512:(s + 1) * 512],
                                        wkv_sb[:, kc, 0:KF],
                                        xT[kc][:, h * 1024 + s * 512:
                                               h * 1024 + (s + 1) * 512],
                                        start=st, stop=sp)
                            nc.tensor.matmul(q0a[:], wq_sb[:, kc, 0:128],
                                             xT[kc][:, 0:512], start=st, stop=sp)
                            nc.tensor.matmul(q0b[:], wq_sb[:, kc, 0:128],
                                             xT[kc][:, 512:1024],
                                             start=st, stop=sp)

                    # rope K halves + Q0 left (rot tiles in a fresh 2-bank pool)
                    with tc.tile_pool(name="phA_rot1", bufs=1,
                                      space="PSUM") as pr:
                        for h in range(2):
                            ku = pa.tile([128, 1024], BF16, name="u", tag="u",
                                         bufs=3)
                            nc.scalar.copy(ku[:], kacc[h][:])
                            for s in range(2):
                                rope_span(kT, 0, ku, h * 1024, s * 512, pr)
                        q0u = pa.tile([128, 1024], BF16, name="u", tag="u",
                                      bufs=3)
                        nc.scalar.copy(q0u[:, 0:512], q0a[:])
                        nc.scalar.copy(q0u[:, 512:1024], q0b[:])
                        for s in range(2):
                            rope_span(qT[0], 0, q0u, 0, s * 512, pr)

                # ---- psum scope 2: Q0 right, Q1..Q3, V -------------------
                with tc.tile_pool(name="phA_ps2", bufs=1, space="PSUM") as pc:

                    def accumulate(col0_w, wsb, half):
                        acc = pc.tile([128, 1024], F32, name="acc", tag="acc",
                                      bufs=2)
                        for kc in range(KC):
                            for s in range(2):
                                nc.tensor.matmul(
                                    acc[:, s * 512:(s + 1) * 512],
                                    wsb[:, kc, col0_w:col0_w + 128],
                                    xT[kc][:, half * 1024 + s * 512:
                                           half * 1024 + (s + 1) * 512],
                                    start=(kc == 0), stop=(kc == KC - 1))
                        return acc

                    todo = [(0, 1)] + [(j, h) for j in (1, 2, 3)
                                       for h in (0, 1)]
                    for j, h in todo:
                        acc = accumulate(j * 128, wq_sb, h)
                        qu = pa.tile([128, 1024], BF16, name="u", tag="u", bufs=3)
                        nc.scalar.copy(qu[:], acc[:])
                        for s in range(2):
                            rope_span(qT[j], 0, qu, h * 1024, s * 512, pc)

                    for h in range(2):
                        acc = accumulate(KF, wkv_sb, h)
                        vu = pa.tile([128, 1024], BF16, name="u", tag="u", bufs=3)
                        nc.scalar.copy(vu[:], acc[:])
                        for g in range(2):
                            vtr = pc.tile([128, 512], BF16, name="vtr",
                                          tag="rot", bufs=2)
                            for i in range(8):
                                nc.tensor.transpose(
                                    vtr[:, i * HD:(i + 1) * HD],
                                    vu[g * HD:(g + 1) * HD,
                                       i * 128:(i + 1) * 128],
                                    ident[g * HD:(g + 1) * HD,
                                          g * HD:(g + 1) * HD])
                            nc.vector.tensor_copy(
                                vo[g][:, h * 8:(h + 1) * 8, 0:HD],
                                vtr[:].rearrange("p (t d) -> p t d", d=HD))

                if dbg:
                    nc.sync.dma_start(dbg_d["xt0"][:], xT[0][:])
                    nc.sync.dma_start(dbg_d["qt0"][:], qT[0][:])
                    nc.sync.dma_start(dbg_d["qt1"][:], qT[1][:])
                    nc.sync.dma_start(dbg_d["kt"][:], kT[:])
                    nc.sync.dma_start(dbg_d["cost"][:], cosT[:])
                    nc.sync.dma_start(dbg_d["sint"][:], sinT[:])
                    for g in range(2):
                        nc.sync.dma_start(
                            dbg_d[f"vo{g}"][:, 0:NT * (HD + 1)],
                            vo[g][:].rearrange("p t d -> p (t d)"))

            # ================= phase B: attention =========================
            with tc.tile_pool(name="ps_sc", bufs=1, space="PSUM") as ps_sc, \
                 tc.tile_pool(name="ps_cx", bufs=1, space="PSUM") as ps_cx, \
                 tc.tile_pool(name="dramn", bufs=2, space="DRAM") as dnp, \
                 tc.tile_pool(name="attnp", bufs=1) as ap_:

                # flat job list: one job per (head, m, psc tile)
                jobs = []
                for l in range(8):
                    for m in range(NT):
                        c0 = m * 128
                        for t in range(c0 // 1024, 2):
                            jobs.append({
                                "l": l, "m": m, "t": t,
                                "a": max(c0, t * 1024), "b": (t + 1) * 1024,
                            })
                psx_state = {}   # head -> [psx0, psx1]

                def emit_scores(job, exp_idx):
                    l, m, t = job["l"], job["m"], job["t"]
                    g, chunk = l // 4, l % 4
                    r0 = 64 * g
                    c0 = m * 128
                    a, b = job["a"], job["b"]
                    lhs_k = kT[r0:r0 + 64, c0:c0 + 128]
                    psc = ps_sc.tile([128, 1024], F32, name="psc", tag="psc",
                                     bufs=2)
                    # start=True marks the WHOLE 2KB psum bank pending-zero,
                    # so only the first piece touching each bank may set it.
                    started_banks = set()
                    c = a
                    while c < b:
                        nw = 128 if c == c0 else min(512 - c % 512, b - c)
                        bank = (c - t * 1024) // 512
                        st = bank not in started_banks
                        started_banks.add(bank)
                        nc.tensor.matmul(
                            psc[:, c - t * 1024:c - t * 1024 + nw],
                            lhs_k, qT[chunk][r0:r0 + 64, c:c + nw],
                            start=st, stop=(c != c0), skip_group_check=True)
                        c += nw
                    if a == c0:
                        nc.tensor.matmul(
                            psc[:, c0 - t * 1024:c0 - t * 1024 + 128],
                            maskLT[:], ident[:], start=False, stop=True,
                            skip_group_check=True)
                    off0 = a - t * 1024
                    at = ap_.tile([128, 1024], BF16, name="at", tag="at", bufs=3)
                    if EXP_PATTERN[exp_idx % len(EXP_PATTERN)]:
                        nc.vector.tensor_scalar(
                            at[:, off0:1024].bitcast(I16), psc[:, off0:1024],
                            SCALE * EXP_A, EXP_B,
                            mybir.AluOpType.mult, mybir.AluOpType.add)
                    else:
                        nc.scalar.activation(
                            at[:, off0:1024], psc[:, off0:1024],
                            mybir.ActivationFunctionType.Exp, scale=SCALE)
                    job["at"] = at
                    if dbg and l == 0 and m == 0 and t == 0:
                        nc.sync.dma_start(dbg_d["at00"][:, 0:1024], at[:])

                def emit_ctx(job):
                    l, m, t = job["l"], job["m"], job["t"]
                    g, chunk = l // 4, l % 4
                    c0 = m * 128
                    a, b = job["a"], job["b"]
                    if l not in psx_state:
                        psx_state[l] = [None, None]
                    if psx_state[l][t] is None:
                        psx_state[l][t] = ps_cx.tile(
                            [HD + 1, 1024], F32, name="psx", tag="psx", bufs=2)
                    psx = psx_state[l][t]
                    at = job["at"]
                    c = a
                    while c < b:
                        if c == c0:
                            nw, stop = 128, True
                        else:
                            nw, stop = min(512 - c % 512, b - c), False
                        # first touch of each bank (m==0, bank-aligned piece)
                        st = (m == 0) and ((c - t * 1024) % 512 == 0)
                        nc.tensor.matmul(
                            psx[:, c - t * 1024:c - t * 1024 + nw],
                            vo[g][:, m, :],
                            at[:, c - t * 1024:c - t * 1024 + nw],
                            start=st, stop=stop, skip_group_check=True)
                        c += nw
                    # normalize a completed half
                    if (m == 7 and t == 0) or (m == 15 and t == 1):
                        if dbg and l == 0 and t == 0:
                            pscp = ap_.tile([HD + 1, 1024], BF16, name="pscp")
                            nc.vector.tensor_copy(pscp[:], psx[:])
                            nc.sync.dma_start(
                                dbg_d["psx0"][0:HD + 1, 0:1024], pscp[:])
                        ds = ap_.tile([HD + 1, 1024], F32, name="ds", tag="ds",
                                      bufs=2)
                        # move denominators out of PSUM, broadcast RAW, then
                        # approx-recip on the base-0 multi-partition tile
                        # (the custom DVE op mishandles PSUM sources and
                        # non-zero base partitions).
                        nc.scalar.copy(ds[HD:HD + 1, :], psx[HD:HD + 1, :])
                        rd = dnp.tile([1, 1024], F32, name="rd", tag="rd",
                                      bufs=2)
                        nc.sync.dma_start(rd[:], ds[HD:HD + 1, :])
                        rbr = ap_.tile([HD, 1024], F32, name="rbr", tag="rbr",
                                       bufs=2)
                        nc.sync.dma_start(
                            rbr[:], rd[:].to_broadcast((HD, 1024)))
                        rb = ap_.tile([HD, 1024], F32, name="rb", tag="rb",
                                      bufs=2)
                        nc.vector.reciprocal_approx_fast(rb[:], rbr[:])
                        if dbg and l == 0 and t == 0:
                            nc.gpsimd.dma_start(
                                dbg_d["nds"][0:1, 0:1024], ds[HD:HD + 1, :])
                            nc.gpsimd.dma_start(
                                dbg_d["nds2"][0:1, 0:1024], rb[0:1, :])
                            nc.gpsimd.dma_start(
                                dbg_d["nrb"][0:HD, 0:1024], rb[:])
                        if l < 4:
                            nc.vector.tensor_mul(
                                ctxT[chunk][0:HD, t * 1024:(t + 1) * 1024],
                                psx[0:HD, :], rb[:])
                        else:
                            stg = ap_.tile([HD, 1024], BF16, name="stg",
                                           tag="stg", bufs=2)
                            nc.vector.tensor_mul(stg[:], psx[0:HD, :], rb[:])
                            nc.sync.dma_start(
                                ctxT[chunk][HD:128, t * 1024:(t + 1) * 1024],
                                stg[:])
                        psx_state[l][t] = None

                # software pipeline: ctx delayed one job behind scores/exp
                import os as _os
                if _os.environ.get("KERNEL_NO_PIPELINE"):
                    for ji, job in enumerate(jobs):
                        emit_scores(job, ji)
                        emit_ctx(job)
                else:
                    for ji, job in enumerate(jobs):
                        emit_scores(job, ji)
                        if ji > 0:
                            emit_ctx(jobs[ji - 1])
                    emit_ctx(jobs[-1])
                if dbg:
                    nc.sync.dma_start(dbg_d["ctxt0"][:], ctxT[0][:])
                    nc.sync.dma_start(dbg_d["ctxt1"][:], ctxT[1][:])

            # ================= phase C: output projection =================
            with tc.tile_pool(name="ps_o", bufs=1, space="PSUM") as ps_o, \
                 tc.tile_pool(name="outp", bufs=1) as op_:
                for oc in range(NT):
                    pso = ps_o.tile([128, N], F32, name="pso", tag="pso", bufs=2)
                    for k4 in range(4):
                        lhsT = wo_sb[:, k4, oc * 128:(oc + 1) * 128]
                        for s in range(4):
                            nc.tensor.matmul(
                                pso[:, s * 512:(s + 1) * 512], lhsT,
                                ctxT[k4][:, s * 512:(s + 1) * 512],
                                start=(k4 == 0), stop=(k4 == 3))
                    ob = op_.tile([128, N], BF16, name="ob", tag="ob", bufs=2)
                    if oc % 2 == 0:
                        nc.scalar.copy(ob[:], pso[:])
                    else:
                        nc.vector.tensor_copy(ob[:], pso[:])
                    nc.sync.dma_start(out_d[oc * 128:(oc + 1) * 128, :], ob[:])

    nc.compile()
    return nc


_NC_CACHE = {}


def _get_nc():
    if "nc" not in _NC_CACHE:
        _NC_CACHE["nc"] = _build_program()
    return _NC_CACHE["nc"]


# local-head permutation: chunk j holds [head j | head j+4]
_PERM = np.concatenate(
    [np.arange(j * HD, (j + 1) * HD) for pair in range(4)
     for j in (pair, pair + 4)])


def kernel(x, cos, sin, mask, Wq, Wk, Wv, Wo, _trace=False, _trace_kwargs=None):
    x = np.asarray(x, dtype=np.float32)
    cos = np.asarray(cos, dtype=np.float32)
    sin = np.asarray(sin, dtype=np.float32)
    Wq = np.asarray(Wq, dtype=np.float32)
    Wk = np.asarray(Wk, dtype=np.float32)
    Wv = np.asarray(Wv, dtype=np.float32)
    Wo = np.asarray(Wo, dtype=np.float32)

    nc = _get_nc()
    in_maps = []
    for c in range(8):
        bidx = c // 4
        p = c % 4
        wq_l = Wq[:, p * QF:(p + 1) * QF][:, _PERM]
        wo_l = Wo[p * QF:(p + 1) * QF, :][_PERM, :]
        in_maps.append({
            "x": np.ascontiguousarray(x[bidx]),
            "cos": cos,
            "sin": sin,
            "wq": np.ascontiguousarray(wq_l),
            "wk": np.ascontiguousarray(Wk[:, p * KF:(p + 1) * KF]),
            "wv": np.ascontiguousarray(Wv[:, p * KF:(p + 1) * KF]),
            "wo": np.ascontiguousarray(wo_l),
        })

    kwargs = {}
    if _trace:
        kwargs["trace"] = True
        kwargs.update(_trace_kwargs or {})
    res = run_bass_kernel_spmd(nc, in_maps, core_ids=list(range(8)), **kwargs)
    parts = [np.asarray(r["out"], dtype=np.float32) for r in res.results]
    out = np.stack([
        (parts[0] + parts[1] + parts[2] + parts[3]).T,
        (parts[4] + parts[5] + parts[6] + parts[7]).T,
    ]).astype(np.float32)
    if _trace:
        kernel._last_result = res
    return out


# revision 29
# speedup vs baseline: 1.4397x; 1.0398x over previous
"""GQA attention kernel for 8 Trainium2 NeuronCores (v2).

Problem: B=2, N=2048, D=2048, H=32 heads, G=8 KV groups, head_dim=64, RoPE,
causal mask, fused QKV/output projections.

Sharding: one (batch, group-pair) unit per core - core c handles batch c//4
and KV groups {2*(c%4), 2*(c%4)+1} (8 query heads). Each core computes a
partial output projection; the host sums 4 partials per batch (and
transposes: the kernel emits out^T [dout, tok] in bf16).

Per-core pipeline (bf16 matmuls, fp32 accumulate):
  x --cast DMA--> xn stripes (SBUF bf16) --PE transpose--> xT [d, tok]
  Projections use stationary-swap: stationary = weight chunk [128d, 128f],
  moving = xT spans -> outputs land TRANSPOSED (qT/kT/vT [feat, tok]).
  RoPE: qrot = rotP @ qU on PE (rotP = signed rotate-half permutation), then
  qT = qU*cosT + qrot*sinT on DVE (cosT/sinT pre-transposed [hd, tok]).
  Attention per head, key-block-major, software-pipelined one tile deep;
  the causal mask is ADDED on PE via an extra matmul (maskLT^T @ I) into
  the scores PSUM; exp is split between ACT (true exp) and DVE (int16
  Schraudolph fast-exp written through a bf16 bitcast); ctx^T accumulates
  in column halves [65, 1024] whose ones-row yields softmax denominators;
  normalize = DVE approx-reciprocal + SBUF broadcast DMA + DVE multiply.
  Output proj: stationary = Wo chunk, moving = ctxT -> out^T, bf16 DMA.

Host-side: Wq columns / Wo rows are permuted so q-head chunk j holds
[head j | head j+4] - this aligns each head's qT partition base (0/64)
with its KV group's kT partition base, as the PE requires.
"""

import numpy as np

import concourse.bass as bass
import concourse.bacc as bacc
import concourse.mybir as mybir
import concourse.tile as tile
from concourse.bass_utils import run_bass_kernel_spmd
from concourse.masks import make_identity, make_upper_triangular

F32 = mybir.dt.float32
BF16 = mybir.dt.bfloat16
I16 = mybir.dt.int16

N = 2048          # sequence length
D = 2048          # model dim
HD = 64           # head dim
QF = 512          # q features per core (8 heads)
KF = 128          # k/v features per core (2 groups)
NT = N // 128     # token blocks
KC = D // 128     # contraction chunks
SCALE = 1.0 / 8.0  # 1/sqrt(HD)
MASKVAL = -480.0   # additive causal mask pre-scale (-60 post-scale)

# bf16 Schraudolph fast-exp: bits = round(x*SCALE*A + B) as int16 -> bf16
EXP_A = 128.0 / float(np.log(2.0))
EXP_B = 16256.0 - 7.0
# exp engine pattern per psc tile (cycled): False=ACT true exp, True=DVE fast
EXP_PATTERN = (False, False, False, True, True)


def _build_program(dbg=False):
    nc = bacc.Bacc("TRN2", debug=False, target_bir_lowering=False)

    x_d = nc.dram_tensor("x", [N, D], F32, kind="ExternalInput")
    cos_d = nc.dram_tensor("cos", [N, HD], F32, kind="ExternalInput")
    sin_d = nc.dram_tensor("sin", [N, HD], F32, kind="ExternalInput")
    wq_d = nc.dram_tensor("wq", [D, QF], F32, kind="ExternalInput")
    wk_d = nc.dram_tensor("wk", [D, KF], F32, kind="ExternalInput")
    wv_d = nc.dram_tensor("wv", [D, KF], F32, kind="ExternalInput")
    wo_d = nc.dram_tensor("wo", [QF, D], F32, kind="ExternalInput")
    out_d = nc.dram_tensor("out", [D, N], BF16, kind="ExternalOutput")
    if dbg:
        dbg_d = {nm: nc.dram_tensor(f"dbg_{nm}", [128, N], BF16,
                                    kind="ExternalOutput")
                 for nm in ("xt0", "qt0", "qt1", "kt", "cost", "sint",
                            "ctxt0", "ctxt1", "vo0", "vo1", "at00", "psx0",
                            "nds", "nds2", "nrb")}

    with tile.TileContext(nc) as tc:
        with tc.tile_pool(name="persist", bufs=1) as pp:
            qT = [pp.tile([128, N], BF16, name=f"qT{j}") for j in range(4)]
            kT = pp.tile([128, N], BF16, name="kT")
            vo = [pp.tile([128, NT, HD + 1], BF16, name=f"vo{g}") for g in range(2)]
            ctxT = [pp.tile([128, N], BF16, name=f"ctxT{j}") for j in range(4)]
            wq_sb = pp.tile([128, KC, QF], BF16, name="wq_sb")
            wkv_sb = pp.tile([128, KC, 2 * KF], BF16, name="wkv_sb")
            wo_sb = pp.tile([128, 4, N], BF16, name="wo_sb")
            cosT = pp.tile([128, N], BF16, name="cosT")
            sinT = pp.tile([128, N], BF16, name="sinT")
            ident = pp.tile([128, 128], BF16, name="ident")
            maskLT = pp.tile([128, 128], BF16, name="maskLT")
            rotP = pp.tile([128, 128], BF16, name="rotP")

            make_identity(nc, ident)
            # maskLT[r, key] = MASKVAL for key > r (strict upper)
            make_upper_triangular(nc, maskLT, val=MASKVAL, diag=False)

            # rotate-half as lhsT: rotP[k, k+32] = +1 (k%64<32),
            #                      rotP[k, k-32] = -1 (k%64>=32)
            nc.gpsimd.memset(rotP[:], 0.0)
            for b0 in (0, 64):
                nc.gpsimd.tensor_copy(
                    rotP[b0:b0 + 32, b0 + 32:b0 + 64],
                    ident[b0:b0 + 32, b0:b0 + 32])
                nc.gpsimd.tensor_scalar_mul(
                    rotP[b0 + 32:b0 + 64, b0:b0 + 32],
                    ident[b0 + 32:b0 + 64, b0 + 32:b0 + 64], -1.0)
            for g in range(2):
                nc.vector.memset(vo[g][:, :, HD:HD + 1], 1.0)

            # ---- DMA issue order: cs, wkv, then wq groups // x stripes ---
            with tc.tile_pool(name="phA_sb", bufs=1) as pa:
                cs_nat = pa.tile([128, NT, 2 * HD], F32, name="cs_nat")
                nc.sync.dma_start(
                    cs_nat[:, :, 0:HD],
                    cos_d[:].rearrange("(t p) d -> p t d", p=128))
                nc.sync.dma_start(
                    cs_nat[:, :, HD:2 * HD],
                    sin_d[:].rearrange("(t p) d -> p t d", p=128))
                nc.gpsimd.dma_start(
                    wkv_sb[:, :, 0:KF],
                    wk_d[:].rearrange("(ko ki) n -> ki ko n", ki=128))
                nc.gpsimd.dma_start(
                    wkv_sb[:, :, KF:2 * KF],
                    wv_d[:].rearrange("(ko ki) n -> ki ko n", ki=128))

                xn = []
                for kc in range(KC):
                    t = pa.tile([128, NT, 128], BF16, name="xn", tag="xn", bufs=4)
                    nc.gpsimd.dma_start(
                        t[:], x_d[:, kc * 128:(kc + 1) * 128]
                        .rearrange("(t p) d -> p t d", p=128))
                    xn.append(t)
                    if kc % 4 == 0:
                        kg = kc // 4
                        nc.gpsimd.dma_start(
                            wq_sb[:, kg * 4:(kg + 1) * 4, :],
                            wq_d[kg * 512:(kg + 1) * 512, :]
                            .rearrange("(ko ki) n -> ki ko n", ki=128))
                nc.gpsimd.dma_start(
                    wo_sb[:], wo_d[:].rearrange("(ko ki) n -> ki ko n", ki=128))

                cs_bf = pa.tile([128, NT, 2 * HD], BF16, name="cs_bf")
                nc.vector.tensor_copy(cs_bf[:], cs_nat[:])

                xT = [pa.tile([128, N], BF16, name=f"xT{kc}") for kc in range(KC)]

                # generic rope over one 512-col span
                def rope_span(dst_tile, row0_u, u, col0, s_off, rot_pool,
                              rot_tag="rot"):
                    """dst_tile cols [col0+s_off, col0+s_off+512) =
                    u[:, s_off:s_off+512]*cosT + (rotP@u)*sinT."""
                    rot = rot_pool.tile([128, 512], F32, name="rot", tag=rot_tag,
                                        bufs=2)
                    nc.tensor.matmul(rot[:], rotP[:], u[:, s_off:s_off + 512],
                                     start=True, stop=True)
                    c0 = col0 + s_off
                    t1 = pa.tile([128, 512], BF16, name="t1", tag="t1", bufs=2)
                    t2 = pa.tile([128, 512], BF16, name="t2", tag="t2", bufs=2)
                    nc.vector.tensor_mul(t1[:], u[:, s_off:s_off + 512],
                                         cosT[:, c0:c0 + 512])
                    nc.vector.tensor_mul(t2[:], rot[:], sinT[:, c0:c0 + 512])
                    nc.vector.tensor_add(dst_tile[:, c0:c0 + 512], t1[:], t2[:])

                # ---- psum scope 1: K (halves) + Q0 left (quarters) -------
                with tc.tile_pool(name="phA_ps1", bufs=1, space="PSUM") as pk:
                    kacc = [pk.tile([128, 1024], F32, name=f"kacc{h}")
                            for h in range(2)]
                    q0a = pk.tile([128, 512], F32, name="q0a")
                    q0b = pk.tile([128, 512], F32, name="q0b")

                    with tc.tile_pool(name="phA_tr", bufs=1, space="PSUM") as pt:
                        # cos/sin transposes (tiny)
                        for tb in range(NT):
                            ptc = pt.tile([128, 128], BF16, name="ptc", tag="tr",
                                          bufs=2)
                            nc.tensor.transpose(
                                ptc[0:2 * HD, :], cs_bf[:, tb, :], ident[:])
                            nc.vector.tensor_copy(
                                cosT[0:HD, tb * 128:(tb + 1) * 128],
                                ptc[0:HD, :])
                            nc.scalar.copy(
                                sinT[0:HD, tb * 128:(tb + 1) * 128],
                                ptc[HD:2 * HD, :])
                        nc.sync.dma_start(cosT[HD:128, :], cosT[0:HD, :])
                        nc.sync.dma_start(sinT[HD:128, :], sinT[0:HD, :])

                        # stripe loop: x transposes + K + Q0-left accumulation
                        for kc in range(KC):
                            for tb4 in range(4):
                                ptr = pt.tile([128, 512], BF16, name="ptr",
                                              tag="tr", bufs=2)
                                for i in range(4):
                                    tb = tb4 * 4 + i
                                    nc.tensor.transpose(
                                        ptr[:, i * 128:(i + 1) * 128],
                                        xn[kc][:, tb, :], ident[:])
                                cp = nc.vector.tensor_copy if tb4 % 2 == 0 \
                                    else nc.scalar.copy
                                cp(xT[kc][:, tb4 * 512:(tb4 + 1) * 512], ptr[:])
                            st = kc == 0
                            sp = kc == KC - 1
                            for h in range(2):
                                for s in range(2):
                                    nc.tensor.matmul(
                                        kacc[h][:, s * 512:(s + 1) * 512],
                                        wkv_sb[:, kc, 0:KF],
                                        xT[kc][:, h * 1024 + s * 512:
                                               h * 1024 + (s + 1) * 512],
                                        start=st, stop=sp)
                            nc.tensor.matmul(q0a[:], wq_sb[:, kc, 0:128],
                                             xT[kc][:, 0:512], start=st, stop=sp)
                            nc.tensor.matmul(q0b[:], wq_sb[:, kc, 0:128],
                                             xT[kc][:, 512:1024],
                                             start=st, stop=sp)

                    # rope K halves + Q0 left (rot tiles in a fresh 2-bank pool)
                    with tc.tile_pool(name="phA_rot1", bufs=1,
                                      space="PSUM") as pr:
                        for h in range(2):
                            ku = pa.tile([128, 1024], BF16, name="u", tag="u",
                                         bufs=3)
                            nc.scalar.copy(ku[:], kacc[h][:])
                            for s in range(2):
                                rope_span(kT, 0, ku, h * 1024, s * 512, pr)
                        q0u = pa.tile([128, 1024], BF16, name="u", tag="u",
                                      bufs=3)
                        nc.scalar.copy(q0u[:, 0:512], q0a[:])
                        nc.scalar.copy(q0u[:, 512:1024], q0b[:])
                        for s in range(2):
                            rope_span(qT[0], 0, q0u, 0, s * 512, pr)

                # ---- psum scope 2: Q0 right, Q1..Q3, V -------------------
                with tc.tile_pool(name="phA_ps2", bufs=1, space="PSUM") as pc:

                    def accumulate(col0_w, wsb, half):
                        acc = pc.tile([128, 1024], F32, name="acc", tag="acc",
                                      bufs=2)
                        for kc in range(KC):
                            for s in range(2):
                                nc.tensor.matmul(
                                    acc[:, s * 512:(s + 1) * 512],
                                    wsb[:, kc, col0_w:col0_w + 128],
                                    xT[kc][:, half * 1024 + s * 512:
                                           half * 1024 + (s + 1) * 512],
                                    start=(kc == 0), stop=(kc == KC - 1))
                        return acc

                    todo = [(0, 1)] + [(j, h) for j in (1, 2, 3)
                                       for h in (0, 1)]
                    for j, h in todo:
                        acc = accumulate(j * 128, wq_sb, h)
                        qu = pa.tile([128, 1024], BF16, name="u", tag="u", bufs=3)
                        nc.scalar.copy(qu[:], acc[:])
                        for s in range(2):
                            rope_span(qT[j], 0, qu, h * 1024, s * 512, pc)

                    for h in range(2):
                        acc = accumulate(KF, wkv_sb, h)
                        vu = pa.tile([128, 1024], BF16, name="u", tag="u", bufs=3)
                        nc.scalar.copy(vu[:], acc[:])
                        for g in range(2):
                            vtr = pc.tile([128, 512], BF16, name="vtr",
                                          tag="rot", bufs=2)
                            for i in range(8):
                                nc.tensor.transpose(
                                    vtr[:, i * HD:(i + 1) * HD],
                                    vu[g * HD:(g + 1) * HD,
                                       i * 128:(i + 1) * 128],
                                    ident[g * HD:(g + 1) * HD,
                                          g * HD:(g + 1) * HD])
                            nc.vector.tensor_copy(
                                vo[g][:, h * 8:(h + 1) * 8, 0:HD],
                                vtr[:].rearrange("p (t d) -> p t d", d=HD))

                if dbg:
                    nc.sync.dma_start(dbg_d["xt0"][:], xT[0][:])
                    nc.sync.dma_start(dbg_d["qt0"][:], qT[0][:])
                    nc.sync.dma_start(dbg_d["qt1"][:], qT[1][:])
                    nc.sync.dma_start(dbg_d["kt"][:], kT[:])
                    nc.sync.dma_start(dbg_d["cost"][:], cosT[:])
                    nc.sync.dma_start(dbg_d["sint"][:], sinT[:])
                    for g in range(2):
                        nc.sync.dma_start(
                            dbg_d[f"vo{g}"][:, 0:NT * (HD + 1)],
                            vo[g][:].rearrange("p t d -> p (t d)"))

            # ================= phase B: attention =========================
            with tc.tile_pool(name="ps_sc", bufs=1, space="PSUM") as ps_sc, \
                 tc.tile_pool(name="ps_cx", bufs=1, space="PSUM") as ps_cx, \
                 tc.tile_pool(name="dramn", bufs=2, space="DRAM") as dnp, \
                 tc.tile_pool(name="attnp", bufs=1) as ap_:

                # flat job list: one job per (head, m, psc tile)
                jobs = []
                for l in range(8):
                    for m in range(NT):
                        c0 = m * 128
                        for t in range(c0 // 1024, 2):
                            jobs.append({
                                "l": l, "m": m, "t": t,
                                "a": max(c0, t * 1024), "b": (t + 1) * 1024,
                            })
                psx_state = {}   # head -> [psx0, psx1]

                def emit_scores(job, exp_idx):
                    l, m, t = job["l"], job["m"], job["t"]
                    g, chunk = l // 4, l % 4
                    r0 = 64 * g
                    c0 = m * 128
                    a, b = job["a"], job["b"]
                    lhs_k = kT[r0:r0 + 64, c0:c0 + 128]
                    psc = ps_sc.tile([128, 1024], F32, name="psc", tag="psc",
                                     bufs=2)
                    # start=True marks the WHOLE 2KB psum bank pending-zero,
                    # so only the first piece touching each bank may set it.
                    started_banks = set()
                    c = a
                    while c < b:
                        nw = 128 if c == c0 else min(512 - c % 512, b - c)
                        bank = (c - t * 1024) // 512
                        st = bank not in started_banks
                        started_banks.add(bank)
                        nc.tensor.matmul(
                            psc[:, c - t * 1024:c - t * 1024 + nw],
                            lhs_k, qT[chunk][r0:r0 + 64, c:c + nw],
                            start=st, stop=(c != c0), skip_group_check=True)
                        c += nw
                    if a == c0:
                        nc.tensor.matmul(
                            psc[:, c0 - t * 1024:c0 - t * 1024 + 128],
                            maskLT[:], ident[:], start=False, stop=True,
                            skip_group_check=True)
                    off0 = a - t * 1024
                    at = ap_.tile([128, 1024], BF16, name="at", tag="at", bufs=3)
                    if EXP_PATTERN[exp_idx % len(EXP_PATTERN)]:
                        nc.vector.tensor_scalar(
                            at[:, off0:1024].bitcast(I16), psc[:, off0:1024],
                            SCALE * EXP_A, EXP_B,
                            mybir.AluOpType.mult, mybir.AluOpType.add)
                    else:
                        nc.scalar.activation(
                            at[:, off0:1024], psc[:, off0:1024],
                            mybir.ActivationFunctionType.Exp, scale=SCALE)
                    job["at"] = at
                    if dbg and l == 0 and m == 0 and t == 0:
                        nc.sync.dma_start(dbg_d["at00"][:, 0:1024], at[:])

                def emit_ctx(job):
                    l, m, t = job["l"], job["m"], job["t"]
                    g, chunk = l // 4, l % 4
                    c0 = m * 128
                    a, b = job["a"], job["b"]
                    if l not in psx_state:
                        psx_state[l] = [None, None]
                    if psx_state[l][t] is None:
                        psx_state[l][t] = ps_cx.tile(
                            [HD + 1, 1024], F32, name="psx", tag="psx", bufs=2)
                    psx = psx_state[l][t]
                    at = job["at"]
                    c = a
                    while c < b:
                        if c == c0:
                            nw, stop = 128, True
                        else:
                            nw, stop = min(512 - c % 512, b - c), False
                        # first touch of each bank (m==0, bank-aligned piece)
                        st = (m == 0) and ((c - t * 1024) % 512 == 0)
                        nc.tensor.matmul(
                            psx[:, c - t * 1024:c - t * 1024 + nw],
                            vo[g][:, m, :],
                            at[:, c - t * 1024:c - t * 1024 + nw],
                            start=st, stop=stop, skip_group_check=True)
                        c += nw
                    # normalize a completed half
                    if (m == 7 and t == 0) or (m == 15 and t == 1):
                        if dbg and l == 0 and t == 0:
                            pscp = ap_.tile([HD + 1, 1024], BF16, name="pscp")
                            nc.vector.tensor_copy(pscp[:], psx[:])
                            nc.sync.dma_start(
                                dbg_d["psx0"][0:HD + 1, 0:1024], pscp[:])
                        ds = ap_.tile([HD + 1, 1024], F32, name="ds", tag="ds",
                                      bufs=2)
                        # move denominators out of PSUM, broadcast RAW, then
                        # approx-recip on the base-0 multi-partition tile
                        # (the custom DVE op mishandles PSUM sources and
                        # non-zero base partitions).
                        nc.scalar.copy(ds[HD:HD + 1, :], psx[HD:HD + 1, :])
                        rd = dnp.tile([1, 1024], F32, name="rd", tag="rd",
                                      bufs=2)
                        nc.sync.dma_start(rd[:], ds[HD:HD + 1, :])
                        rbr = ap_.tile([HD, 1024], F32, name="rbr", tag="rbr",
                                       bufs=2)
                        nc.sync.dma_start(
                            rbr[:], rd[:].to_broadcast((HD, 1024)))
                        rb = ap_.tile([HD, 1024], F32, name="rb", tag="rb",
                                      bufs=2)
                        nc.vector.reciprocal_approx_fast(rb[:], rbr[:])
                        if dbg and l == 0 and t == 0:
                            nc.gpsimd.dma_start(
                                dbg_d["nds"][0:1, 0:1024], ds[HD:HD + 1, :])
                            nc.gpsimd.dma_start(
                                dbg_d["nds2"][0:1, 0:1024], rb[0:1, :])
                            nc.gpsimd.dma_start(
                                dbg_d["nrb"][0:HD, 0:1024], rb[:])
                        if l < 4:
                            nc.vector.tensor_mul(
                                ctxT[chunk][0:HD, t * 1024:(t + 1) * 1024],
                                psx[0:HD, :], rb[:])
                        else:
                            stg = ap_.tile([HD, 1024], BF16, name="stg",
                                           tag="stg", bufs=2)
                            nc.vector.tensor_mul(stg[:], psx[0:HD, :], rb[:])
                            nc.sync.dma_start(
                                ctxT[chunk][HD:128, t * 1024:(t + 1) * 1024],
                                stg[:])
                        psx_state[l][t] = None

                # software pipeline: ctx delayed two jobs behind scores/exp
                # so the PE never stalls on exp latency (at bufs=3, psc
                # bufs=2 bound the depth)
                DELAY = 2
                for ji, job in enumerate(jobs):
                    emit_scores(job, ji)
                    if ji >= DELAY:
                        emit_ctx(jobs[ji - DELAY])
                for job in jobs[-DELAY:]:
                    emit_ctx(job)
                if dbg:
                    nc.sync.dma_start(dbg_d["ctxt0"][:], ctxT[0][:])
                    nc.sync.dma_start(dbg_d["ctxt1"][:], ctxT[1][:])

            # ================= phase C: output projection =================
            with tc.tile_pool(name="ps_o", bufs=1, space="PSUM") as ps_o, \
                 tc.tile_pool(name="outp", bufs=1) as op_:
                for oc in range(NT):
                    pso = ps_o.tile([128, N], F32, name="pso", tag="pso", bufs=2)
                    for k4 in range(4):
                        lhsT = wo_sb[:, k4, oc * 128:(oc + 1) * 128]
                        for s in range(4):
                            nc.tensor.matmul(
                                pso[:, s * 512:(s + 1) * 512], lhsT,
                                ctxT[k4][:, s * 512:(s + 1) * 512],
                                start=(k4 == 0), stop=(k4 == 3))
                    ob = op_.tile([128, N], BF16, name="ob", tag="ob", bufs=2)
                    if oc % 2 == 0:
                        nc.scalar.copy(ob[:], pso[:])
                    else:
                        nc.vector.tensor_copy(ob[:], pso[:])
                    nc.sync.dma_start(out_d[oc * 128:(oc + 1) * 128, :], ob[:])

    nc.compile()
    return nc


_NC_CACHE = {}


def _get_nc():
    if "nc" not in _NC_CACHE:
        _NC_CACHE["nc"] = _build_program()
    return _NC_CACHE["nc"]


# local-head permutation: chunk j holds [head j | head j+4]
_PERM = np.concatenate(
    [np.arange(j * HD, (j + 1) * HD) for pair in range(4)
     for j in (pair, pair + 4)])


def kernel(x, cos, sin, mask, Wq, Wk, Wv, Wo, _trace=False, _trace_kwargs=None):
    x = np.asarray(x, dtype=np.float32)
    cos = np.asarray(cos, dtype=np.float32)
    sin = np.asarray(sin, dtype=np.float32)
    Wq = np.asarray(Wq, dtype=np.float32)
    Wk = np.asarray(Wk, dtype=np.float32)
    Wv = np.asarray(Wv, dtype=np.float32)
    Wo = np.asarray(Wo, dtype=np.float32)

    nc = _get_nc()
    in_maps = []
    for c in range(8):
        bidx = c // 4
        p = c % 4
        wq_l = Wq[:, p * QF:(p + 1) * QF][:, _PERM]
        wo_l = Wo[p * QF:(p + 1) * QF, :][_PERM, :]
        in_maps.append({
            "x": np.ascontiguousarray(x[bidx]),
            "cos": cos,
            "sin": sin,
            "wq": np.ascontiguousarray(wq_l),
            "wk": np.ascontiguousarray(Wk[:, p * KF:(p + 1) * KF]),
            "wv": np.ascontiguousarray(Wv[:, p * KF:(p + 1) * KF]),
            "wo": np.ascontiguousarray(wo_l),
        })

    kwargs = {}
    if _trace:
        kwargs["trace"] = True
        kwargs.update(_trace_kwargs or {})
    res = run_bass_kernel_spmd(nc, in_maps, core_ids=list(range(8)), **kwargs)
    parts = [np.asarray(r["out"], dtype=np.float32) for r in res.results]
    out = np.stack([
        (parts[0] + parts[1] + parts[2] + parts[3]).T,
        (parts[4] + parts[5] + parts[6] + parts[7]).T,
    ]).astype(np.float32)
    if _trace:
        kernel._last_result = res
    return out


# revision 31
# speedup vs baseline: 1.5125x; 1.0506x over previous
"""GQA attention kernel for 8 Trainium2 NeuronCores (v2).

Problem: B=2, N=2048, D=2048, H=32 heads, G=8 KV groups, head_dim=64, RoPE,
causal mask, fused QKV/output projections.

Sharding: one (batch, group-pair) unit per core - core c handles batch c//4
and KV groups {2*(c%4), 2*(c%4)+1} (8 query heads). Each core computes a
partial output projection; the host sums 4 partials per batch (and
transposes: the kernel emits out^T [dout, tok] in bf16).

Per-core pipeline (bf16 matmuls, fp32 accumulate):
  x --cast DMA--> xn stripes (SBUF bf16) --PE transpose--> xT [d, tok]
  Projections use stationary-swap: stationary = weight chunk [128d, 128f],
  moving = xT spans -> outputs land TRANSPOSED (qT/kT/vT [feat, tok]).
  RoPE: qrot = rotP @ qU on PE (rotP = signed rotate-half permutation), then
  qT = qU*cosT + qrot*sinT on DVE (cosT/sinT pre-transposed [hd, tok]).
  Attention per head, key-block-major, software-pipelined one tile deep;
  the causal mask is ADDED on PE via an extra matmul (maskLT^T @ I) into
  the scores PSUM; exp is split between ACT (true exp) and DVE (int16
  Schraudolph fast-exp written through a bf16 bitcast); ctx^T accumulates
  in column halves [65, 1024] whose ones-row yields softmax denominators;
  normalize = DVE approx-reciprocal + SBUF broadcast DMA + DVE multiply.
  Output proj: stationary = Wo chunk, moving = ctxT -> out^T, bf16 DMA.

Host-side: Wq columns / Wo rows are permuted so q-head chunk j holds
[head j | head j+4] - this aligns each head's qT partition base (0/64)
with its KV group's kT partition base, as the PE requires.
"""

import numpy as np

import concourse.bass as bass
import concourse.bacc as bacc
import concourse.mybir as mybir
import concourse.tile as tile
from concourse.bass_utils import run_bass_kernel_spmd
from concourse.masks import make_identity, make_upper_triangular

F32 = mybir.dt.float32
BF16 = mybir.dt.bfloat16
I16 = mybir.dt.int16

N = 2048          # sequence length
D = 2048          # model dim
HD = 64           # head dim
QF = 512          # q features per core (8 heads)
KF = 128          # k/v features per core (2 groups)
NT = N // 128     # token blocks
KC = D // 128     # contraction chunks
SCALE = 1.0 / 8.0  # 1/sqrt(HD)
MASKVAL = -480.0   # additive causal mask pre-scale (-60 post-scale)

# bf16 Schraudolph fast-exp: bits = round(x*SCALE*A + B) as int16 -> bf16
EXP_A = 128.0 / float(np.log(2.0))
EXP_B = 16256.0 - 7.0
# exp engine pattern per psc tile (cycled): False=ACT true exp, True=DVE fast
EXP_PATTERN = (False, False, False, True, True)


def _build_program(dbg=False):
    nc = bacc.Bacc("TRN2", debug=False, target_bir_lowering=False)

    x_d = nc.dram_tensor("x", [N, D], F32, kind="ExternalInput")
    cos_d = nc.dram_tensor("cos", [N, HD], F32, kind="ExternalInput")
    sin_d = nc.dram_tensor("sin", [N, HD], F32, kind="ExternalInput")
    wq_d = nc.dram_tensor("wq", [D, QF], F32, kind="ExternalInput")
    wk_d = nc.dram_tensor("wk", [D, KF], F32, kind="ExternalInput")
    wv_d = nc.dram_tensor("wv", [D, KF], F32, kind="ExternalInput")
    wo_d = nc.dram_tensor("wo", [QF, D], F32, kind="ExternalInput")
    out_d = nc.dram_tensor("out", [D, N], BF16, kind="ExternalOutput")
    if dbg:
        dbg_d = {nm: nc.dram_tensor(f"dbg_{nm}", [128, N], BF16,
                                    kind="ExternalOutput")
                 for nm in ("xt0", "qt0", "qt1", "kt", "cost", "sint",
                            "ctxt0", "ctxt1", "vo0", "vo1", "at00", "psx0",
                            "nds", "nds2", "nrb")}

    with tile.TileContext(nc) as tc:
        with tc.tile_pool(name="persist", bufs=1) as pp:
            qT = [pp.tile([128, N], BF16, name=f"qT{j}") for j in range(4)]
            kT = pp.tile([128, N], BF16, name="kT")
            vo = [pp.tile([128, NT, HD + 1], BF16, name=f"vo{g}") for g in range(2)]
            ctxT = [pp.tile([128, N], BF16, name=f"ctxT{j}") for j in range(4)]
            wq_sb = pp.tile([128, KC, QF], BF16, name="wq_sb")
            wkv_sb = pp.tile([128, KC, 2 * KF], BF16, name="wkv_sb")
            wo_sb = pp.tile([128, 4, N], BF16, name="wo_sb")
            cosT = pp.tile([128, N], BF16, name="cosT")
            sinT = pp.tile([128, N], BF16, name="sinT")
            ident = pp.tile([128, 128], BF16, name="ident")
            maskLT = pp.tile([128, 128], BF16, name="maskLT")
            rotP = pp.tile([128, 128], BF16, name="rotP")

            make_identity(nc, ident)
            # maskLT[r, key] = MASKVAL for key > r (strict upper)
            make_upper_triangular(nc, maskLT, val=MASKVAL, diag=False)

            # rotate-half as lhsT: rotP[k, k+32] = +1 (k%64<32),
            #                      rotP[k, k-32] = -1 (k%64>=32)
            nc.gpsimd.memset(rotP[:], 0.0)
            for b0 in (0, 64):
                nc.gpsimd.tensor_copy(
                    rotP[b0:b0 + 32, b0 + 32:b0 + 64],
                    ident[b0:b0 + 32, b0:b0 + 32])
                nc.gpsimd.tensor_scalar_mul(
                    rotP[b0 + 32:b0 + 64, b0:b0 + 32],
                    ident[b0 + 32:b0 + 64, b0 + 32:b0 + 64], -1.0)
            for g in range(2):
                nc.vector.memset(vo[g][:, :, HD:HD + 1], 1.0)

            # ---- DMA issue order: cs, wkv, then wq groups // x stripes ---
            with tc.tile_pool(name="phA_sb", bufs=1) as pa:
                cs_nat = pa.tile([128, NT, 2 * HD], F32, name="cs_nat")
                nc.sync.dma_start(
                    cs_nat[:, :, 0:HD],
                    cos_d[:].rearrange("(t p) d -> p t d", p=128))
                nc.sync.dma_start(
                    cs_nat[:, :, HD:2 * HD],
                    sin_d[:].rearrange("(t p) d -> p t d", p=128))
                nc.gpsimd.dma_start(
                    wkv_sb[:, :, 0:KF],
                    wk_d[:].rearrange("(ko ki) n -> ki ko n", ki=128))
                nc.gpsimd.dma_start(
                    wkv_sb[:, :, KF:2 * KF],
                    wv_d[:].rearrange("(ko ki) n -> ki ko n", ki=128))

                xn = []
                for kc in range(KC):
                    t = pa.tile([128, NT, 128], BF16, name="xn", tag="xn", bufs=4)
                    nc.gpsimd.dma_start(
                        t[:], x_d[:, kc * 128:(kc + 1) * 128]
                        .rearrange("(t p) d -> p t d", p=128))
                    xn.append(t)
                    if kc % 4 == 0:
                        kg = kc // 4
                        nc.gpsimd.dma_start(
                            wq_sb[:, kg * 4:(kg + 1) * 4, :],
                            wq_d[kg * 512:(kg + 1) * 512, :]
                            .rearrange("(ko ki) n -> ki ko n", ki=128))
                nc.gpsimd.dma_start(
                    wo_sb[:], wo_d[:].rearrange("(ko ki) n -> ki ko n", ki=128))

                cs_bf = pa.tile([128, NT, 2 * HD], BF16, name="cs_bf")
                nc.vector.tensor_copy(cs_bf[:], cs_nat[:])

                xT = [pa.tile([128, N], BF16, name=f"xT{kc}") for kc in range(KC)]

                # generic rope over one 512-col span
                def rope_span(dst_tile, row0_u, u, col0, s_off, rot_pool,
                              rot_tag="rot"):
                    """dst_tile cols [col0+s_off, col0+s_off+512) =
                    u[:, s_off:s_off+512]*cosT + (rotP@u)*sinT."""
                    rot = rot_pool.tile([128, 512], F32, name="rot", tag=rot_tag,
                                        bufs=2)
                    nc.tensor.matmul(rot[:], rotP[:], u[:, s_off:s_off + 512],
                                     start=True, stop=True)
                    c0 = col0 + s_off
                    t1 = pa.tile([128, 512], BF16, name="t1", tag="t1", bufs=2)
                    t2 = pa.tile([128, 512], BF16, name="t2", tag="t2", bufs=2)
                    nc.vector.tensor_mul(t1[:], u[:, s_off:s_off + 512],
                                         cosT[:, c0:c0 + 512])
                    nc.vector.tensor_mul(t2[:], rot[:], sinT[:, c0:c0 + 512])
                    nc.vector.tensor_add(dst_tile[:, c0:c0 + 512], t1[:], t2[:])

                # ---- psum scope 1: K (halves) + Q0 left (quarters) -------
                with tc.tile_pool(name="phA_ps1", bufs=1, space="PSUM") as pk:
                    kacc = [pk.tile([128, 1024], F32, name=f"kacc{h}")
                            for h in range(2)]
                    q0a = pk.tile([128, 512], F32, name="q0a")
                    q0b = pk.tile([128, 512], F32, name="q0b")

                    with tc.tile_pool(name="phA_tr", bufs=1, space="PSUM") as pt:
                        # cos/sin transposes (tiny)
                        for tb in range(NT):
                            ptc = pt.tile([128, 128], BF16, name="ptc", tag="tr",
                                          bufs=2)
                            nc.tensor.transpose(
                                ptc[0:2 * HD, :], cs_bf[:, tb, :], ident[:])
                            nc.vector.tensor_copy(
                                cosT[0:HD, tb * 128:(tb + 1) * 128],
                                ptc[0:HD, :])
                            nc.scalar.copy(
                                sinT[0:HD, tb * 128:(tb + 1) * 128],
                                ptc[HD:2 * HD, :])
                        nc.sync.dma_start(cosT[HD:128, :], cosT[0:HD, :])
                        nc.sync.dma_start(sinT[HD:128, :], sinT[0:HD, :])

                        # stripe loop: x transposes + K + Q0-left accumulation
                        for kc in range(KC):
                            for tb4 in range(4):
                                ptr = pt.tile([128, 512], BF16, name="ptr",
                                              tag="tr", bufs=2)
                                for i in range(4):
                                    tb = tb4 * 4 + i
                                    nc.tensor.transpose(
                                        ptr[:, i * 128:(i + 1) * 128],
                                        xn[kc][:, tb, :], ident[:])
                                cp = nc.vector.tensor_copy if tb4 % 2 == 0 \
                                    else nc.scalar.copy
                                cp(xT[kc][:, tb4 * 512:(tb4 + 1) * 512], ptr[:])
                            st = kc == 0
                            sp = kc == KC - 1
                            for h in range(2):
                                for s in range(2):
                                    nc.tensor.matmul(
                                        kacc[h][:, s * 512:(s + 1) * 512],
                                        wkv_sb[:, kc, 0:KF],
                                        xT[kc][:, h * 1024 + s * 512:
                                               h * 1024 + (s + 1) * 512],
                                        start=st, stop=sp)
                            nc.tensor.matmul(q0a[:], wq_sb[:, kc, 0:128],
                                             xT[kc][:, 0:512], start=st, stop=sp)
                            nc.tensor.matmul(q0b[:], wq_sb[:, kc, 0:128],
                                             xT[kc][:, 512:1024],
                                             start=st, stop=sp)

                    # rope K halves + Q0 left (rot tiles in a fresh 2-bank pool)
                    with tc.tile_pool(name="phA_rot1", bufs=1,
                                      space="PSUM") as pr:
                        for h in range(2):
                            ku = pa.tile([128, 1024], BF16, name="u", tag="u",
                                         bufs=3)
                            nc.scalar.copy(ku[:], kacc[h][:])
                            for s in range(2):
                                rope_span(kT, 0, ku, h * 1024, s * 512, pr)
                        q0u = pa.tile([128, 1024], BF16, name="u", tag="u",
                                      bufs=3)
                        nc.scalar.copy(q0u[:, 0:512], q0a[:])
                        nc.scalar.copy(q0u[:, 512:1024], q0b[:])
                        for s in range(2):
                            rope_span(qT[0], 0, q0u, 0, s * 512, pr)

                # ---- psum scope 2: Q0 right, Q1..Q3, V -------------------
                with tc.tile_pool(name="phA_ps2", bufs=1, space="PSUM") as pc:

                    def accumulate(col0_w, wsb, half):
                        acc = pc.tile([128, 1024], F32, name="acc", tag="acc",
                                      bufs=2)
                        for kc in range(KC):
                            for s in range(2):
                                nc.tensor.matmul(
                                    acc[:, s * 512:(s + 1) * 512],
                                    wsb[:, kc, col0_w:col0_w + 128],
                                    xT[kc][:, half * 1024 + s * 512:
                                           half * 1024 + (s + 1) * 512],
                                    start=(kc == 0), stop=(kc == KC - 1))
                        return acc

                    # delay each chunk's rope PE-ops one iteration so the
                    # rot matmul never stalls the PE on the ACT u-copy
                    todo = [(0, 1)] + [(j, h) for j in (1, 2, 3)
                                       for h in (0, 1)]
                    pending = None
                    for j, h in todo:
                        acc = accumulate(j * 128, wq_sb, h)
                        qu = pa.tile([128, 1024], BF16, name="u", tag="u", bufs=3)
                        nc.scalar.copy(qu[:], acc[:])
                        if pending is not None:
                            pending()
                        pending = (lambda j=j, h=h, qu=qu: [
                            rope_span(qT[j], 0, qu, h * 1024, s * 512, pc)
                            for s in range(2)])

                    for h in range(2):
                        acc = accumulate(KF, wkv_sb, h)
                        vu = pa.tile([128, 1024], BF16, name="u", tag="u", bufs=3)
                        nc.scalar.copy(vu[:], acc[:])
                        if pending is not None:
                            pending()
                            pending = None
                        for g in range(2):
                            vtr = pc.tile([128, 512], BF16, name="vtr",
                                          tag="rot", bufs=2)
                            for i in range(8):
                                nc.tensor.transpose(
                                    vtr[:, i * HD:(i + 1) * HD],
                                    vu[g * HD:(g + 1) * HD,
                                       i * 128:(i + 1) * 128],
                                    ident[g * HD:(g + 1) * HD,
                                          g * HD:(g + 1) * HD])
                            nc.vector.tensor_copy(
                                vo[g][:, h * 8:(h + 1) * 8, 0:HD],
                                vtr[:].rearrange("p (t d) -> p t d", d=HD))

                if dbg:
                    nc.sync.dma_start(dbg_d["xt0"][:], xT[0][:])
                    nc.sync.dma_start(dbg_d["qt0"][:], qT[0][:])
                    nc.sync.dma_start(dbg_d["qt1"][:], qT[1][:])
                    nc.sync.dma_start(dbg_d["kt"][:], kT[:])
                    nc.sync.dma_start(dbg_d["cost"][:], cosT[:])
                    nc.sync.dma_start(dbg_d["sint"][:], sinT[:])
                    for g in range(2):
                        nc.sync.dma_start(
                            dbg_d[f"vo{g}"][:, 0:NT * (HD + 1)],
                            vo[g][:].rearrange("p t d -> p (t d)"))

            # ================= phase B: attention =========================
            with tc.tile_pool(name="ps_sc", bufs=1, space="PSUM") as ps_sc, \
                 tc.tile_pool(name="ps_cx", bufs=1, space="PSUM") as ps_cx, \
                 tc.tile_pool(name="dramn", bufs=2, space="DRAM") as dnp, \
                 tc.tile_pool(name="attnp", bufs=1) as ap_:

                # flat job list: one job per (head, m, psc tile)
                jobs = []
                for l in range(8):
                    for m in range(NT):
                        c0 = m * 128
                        for t in range(c0 // 1024, 2):
                            jobs.append({
                                "l": l, "m": m, "t": t,
                                "a": max(c0, t * 1024), "b": (t + 1) * 1024,
                            })
                psx_state = {}   # head -> [psx0, psx1]

                def emit_scores(job, exp_idx):
                    l, m, t = job["l"], job["m"], job["t"]
                    g, chunk = l // 4, l % 4
                    r0 = 64 * g
                    c0 = m * 128
                    a, b = job["a"], job["b"]
                    lhs_k = kT[r0:r0 + 64, c0:c0 + 128]
                    psc = ps_sc.tile([128, 1024], F32, name="psc", tag="psc",
                                     bufs=2)
                    # start=True marks the WHOLE 2KB psum bank pending-zero,
                    # so only the first piece touching each bank may set it.
                    started_banks = set()
                    c = a
                    while c < b:
                        nw = 128 if c == c0 else min(512 - c % 512, b - c)
                        bank = (c - t * 1024) // 512
                        st = bank not in started_banks
                        started_banks.add(bank)
                        nc.tensor.matmul(
                            psc[:, c - t * 1024:c - t * 1024 + nw],
                            lhs_k, qT[chunk][r0:r0 + 64, c:c + nw],
                            start=st, stop=(c != c0), skip_group_check=True)
                        c += nw
                    if a == c0:
                        nc.tensor.matmul(
                            psc[:, c0 - t * 1024:c0 - t * 1024 + 128],
                            maskLT[:], ident[:], start=False, stop=True,
                            skip_group_check=True)
                    off0 = a - t * 1024
                    at = ap_.tile([128, 1024], BF16, name="at", tag="at", bufs=4)
                    if EXP_PATTERN[exp_idx % len(EXP_PATTERN)]:
                        nc.vector.tensor_scalar(
                            at[:, off0:1024].bitcast(I16), psc[:, off0:1024],
                            SCALE * EXP_A, EXP_B,
                            mybir.AluOpType.mult, mybir.AluOpType.add)
                    else:
                        nc.scalar.activation(
                            at[:, off0:1024], psc[:, off0:1024],
                            mybir.ActivationFunctionType.Exp, scale=SCALE)
                    job["at"] = at
                    if dbg and l == 0 and m == 0 and t == 0:
                        nc.sync.dma_start(dbg_d["at00"][:, 0:1024], at[:])

                def emit_ctx(job):
                    l, m, t = job["l"], job["m"], job["t"]
                    g, chunk = l // 4, l % 4
                    c0 = m * 128
                    a, b = job["a"], job["b"]
                    if l not in psx_state:
                        psx_state[l] = [None, None]
                    if psx_state[l][t] is None:
                        psx_state[l][t] = ps_cx.tile(
                            [HD + 1, 1024], F32, name="psx", tag="psx", bufs=2)
                    psx = psx_state[l][t]
                    at = job["at"]
                    c = a
                    while c < b:
                        if c == c0:
                            nw, stop = 128, True
                        else:
                            nw, stop = min(512 - c % 512, b - c), False
                        # first touch of each bank (m==0, bank-aligned piece)
                        st = (m == 0) and ((c - t * 1024) % 512 == 0)
                        nc.tensor.matmul(
                            psx[:, c - t * 1024:c - t * 1024 + nw],
                            vo[g][:, m, :],
                            at[:, c - t * 1024:c - t * 1024 + nw],
                            start=st, stop=stop, skip_group_check=True)
                        c += nw
                    # normalize a completed half
                    if (m == 7 and t == 0) or (m == 15 and t == 1):
                        if dbg and l == 0 and t == 0:
                            pscp = ap_.tile([HD + 1, 1024], BF16, name="pscp")
                            nc.vector.tensor_copy(pscp[:], psx[:])
                            nc.sync.dma_start(
                                dbg_d["psx0"][0:HD + 1, 0:1024], pscp[:])
                        ds = ap_.tile([HD + 1, 1024], F32, name="ds", tag="ds",
                                      bufs=2)
                        # move denominators out of PSUM, broadcast RAW, then
                        # approx-recip on the base-0 multi-partition tile
                        # (the custom DVE op mishandles PSUM sources and
                        # non-zero base partitions).
                        nc.scalar.copy(ds[HD:HD + 1, :], psx[HD:HD + 1, :])
                        rd = dnp.tile([1, 1024], F32, name="rd", tag="rd",
                                      bufs=2)
                        nc.sync.dma_start(rd[:], ds[HD:HD + 1, :])
                        rbr = ap_.tile([HD, 1024], F32, name="rbr", tag="rbr",
                                       bufs=2)
                        nc.sync.dma_start(
                            rbr[:], rd[:].to_broadcast((HD, 1024)))
                        rb = ap_.tile([HD, 1024], F32, name="rb", tag="rb",
                                      bufs=2)
                        nc.vector.reciprocal_approx_fast(rb[:], rbr[:])
                        if dbg and l == 0 and t == 0:
                            nc.gpsimd.dma_start(
                                dbg_d["nds"][0:1, 0:1024], ds[HD:HD + 1, :])
                            nc.gpsimd.dma_start(
                                dbg_d["nds2"][0:1, 0:1024], rb[0:1, :])
                            nc.gpsimd.dma_start(
                                dbg_d["nrb"][0:HD, 0:1024], rb[:])
                        if l < 4:
                            nc.vector.tensor_mul(
                                ctxT[chunk][0:HD, t * 1024:(t + 1) * 1024],
                                psx[0:HD, :], rb[:])
                        else:
                            stg = ap_.tile([HD, 1024], BF16, name="stg",
                                           tag="stg", bufs=2)
                            nc.vector.tensor_mul(stg[:], psx[0:HD, :], rb[:])
                            nc.sync.dma_start(
                                ctxT[chunk][HD:128, t * 1024:(t + 1) * 1024],
                                stg[:])
                        psx_state[l][t] = None

                # software pipeline: ctx delayed two jobs behind scores/exp
                # so the PE never stalls on exp latency (at bufs=3, psc
                # bufs=2 bound the depth)
                DELAY = 3
                for ji, job in enumerate(jobs):
                    emit_scores(job, ji)
                    if ji >= DELAY:
                        emit_ctx(jobs[ji - DELAY])
                for job in jobs[-DELAY:]:
                    emit_ctx(job)
                if dbg:
                    nc.sync.dma_start(dbg_d["ctxt0"][:], ctxT[0][:])
                    nc.sync.dma_start(dbg_d["ctxt1"][:], ctxT[1][:])

            # ================= phase C: output projection =================
            with tc.tile_pool(name="ps_o", bufs=1, space="PSUM") as ps_o, \
                 tc.tile_pool(name="outp", bufs=1) as op_:
                for oc in range(NT):
                    pso = ps_o.tile([128, N], F32, name="pso", tag="pso", bufs=2)
                    for k4 in range(4):
                        lhsT = wo_sb[:, k4, oc * 128:(oc + 1) * 128]
                        for s in range(4):
                            nc.tensor.matmul(
                                pso[:, s * 512:(s + 1) * 512], lhsT,
                                ctxT[k4][:, s * 512:(s + 1) * 512],
                                start=(k4 == 0), stop=(k4 == 3))
                    ob = op_.tile([128, N], BF16, name="ob", tag="ob", bufs=2)
                    if oc % 2 == 0:
                        nc.scalar.copy(ob[:], pso[:])
                    else:
                        nc.vector.tensor_copy(ob[:], pso[:])
                    nc.sync.dma_start(out_d[oc * 128:(oc + 1) * 128, :], ob[:])

    nc.compile()
    return nc


_NC_CACHE = {}


def _get_nc():
    if "nc" not in _NC_CACHE:
        _NC_CACHE["nc"] = _build_program()
    return _NC_CACHE["nc"]


# local-head permutation: chunk j holds [head j | head j+4]
_PERM = np.concatenate(
    [np.arange(j * HD, (j + 1) * HD) for pair in range(4)
     for j in (pair, pair + 4)])


def kernel(x, cos, sin, mask, Wq, Wk, Wv, Wo, _trace=False, _trace_kwargs=None):
    x = np.asarray(x, dtype=np.float32)
    cos = np.asarray(cos, dtype=np.float32)
    sin = np.asarray(sin, dtype=np.float32)
    Wq = np.asarray(Wq, dtype=np.float32)
    Wk = np.asarray(Wk, dtype=np.float32)
    Wv = np.asarray(Wv, dtype=np.float32)
    Wo = np.asarray(Wo, dtype=np.float32)

    nc = _get_nc()
    in_maps = []
    for c in range(8):
        bidx = c // 4
        p = c % 4
        wq_l = Wq[:, p * QF:(p + 1) * QF][:, _PERM]
        wo_l = Wo[p * QF:(p + 1) * QF, :][_PERM, :]
        in_maps.append({
            "x": np.ascontiguousarray(x[bidx]),
            "cos": cos,
            "sin": sin,
            "wq": np.ascontiguousarray(wq_l),
            "wk": np.ascontiguousarray(Wk[:, p * KF:(p + 1) * KF]),
            "wv": np.ascontiguousarray(Wv[:, p * KF:(p + 1) * KF]),
            "wo": np.ascontiguousarray(wo_l),
        })

    kwargs = {}
    if _trace:
        kwargs["trace"] = True
        kwargs.update(_trace_kwargs or {})
    res = run_bass_kernel_spmd(nc, in_maps, core_ids=list(range(8)), **kwargs)
    parts = [np.asarray(r["out"], dtype=np.float32) for r in res.results]
    out = np.stack([
        (parts[0] + parts[1] + parts[2] + parts[3]).T,
        (parts[4] + parts[5] + parts[6] + parts[7]).T,
    ]).astype(np.float32)
    if _trace:
        kernel._last_result = res
    return out
